# revision 1
# baseline (speedup 1.0000x reference)
"""Trainium2 Bass kernel for a 2-layer LSTM (64, 32) + MLP head.

Model (PyTorch semantics, eval mode):
    h1 = LSTM(4 -> 64)(x)            x: [B=4096, T=512, 4]
    h2 = LSTM(64 -> 32)(h1)
    y  = (relu(h2[:, -1] @ w_fc1.T + b_fc1)) @ w_fc2.T + b_fc2   # [B, 1]

Sharding: data-parallel over batch across 8 NeuronCores (512 rows each),
weights replicated. Inside each core the state is kept *transposed*
([units, batch]) so the per-timestep recurrent matmuls have batch on the
moving free dimension (N=512) and the gate nonlinearities run as a few
wide ops on full 96-partition stacks (layer-1 and layer-2 gates stacked).

State tile S [97, 512]: rows 0:64 = h1^T, rows 64:96 = h2^T, row 96 =
ones (bias row).  Both layers' recurrent matmuls use rhs S[0:97] (base
partition 0 — the PE moving operand must start at 0 to span >32
partitions); layer-1's weight rows over the h2 region are zeros, and
layer-2's over nothing (it genuinely uses h1+h2).  Biases ride the
ones-row through the matmul (incl. the fc1 bias in the head).
The input projection is a K=4 matmul per gate accumulating into the
same PSUM bank; x_t arrives per step by DMA into a small [4, 512]
rotating tile (x is recurrence-independent, so these prefetch ahead and
the matmuls run one step early, filling the TensorE pipe during the
previous step's ACT/DVE chain).

Each gate gets its own PSUM tile (per-tile dependency tracking lets each
sigmoid start as soon as its own gate's matmuls finish); layer-1 and
layer-2 are fused into one M=96 matmul per gate (both contract the same
rhs S[0:97]).  Gate order f,i,g,o: sigmoid(f) (and f*c) overlap the
remaining matmuls; sigmoid(o) fills the ACT gap while the vector engine
runs the cell update; tanh(c) and h close the serial chain.
"""

import numpy as np
from contextlib import ExitStack

import concourse.bass as bass
import concourse.tile as tile
from concourse import bacc, mybir
from concourse import bass_utils

AF = mybir.ActivationFunctionType

B, T, D_IN, H1, H2 = 4096, 512, 4, 64, 32
NCORES = 8
BL = B // NCORES  # 512 batch rows per core

F32 = mybir.dt.float32
# Compute dtypes (flip for perf/accuracy trades):
DT = mybir.dt.bfloat16  # weights / state / gate-activation dtype
CDT = mybir.dt.bfloat16  # cell-state dtype

HS = H1 + H2  # 96: stacked (layer1, layer2) partition extent


def _build(n_steps: int = T):
    """Build the SPMD single-core Bass program (same NEFF on all 8 cores)."""
    nc = bacc.Bacc("TRN2", target_bir_lowering=False, debug=False)

    xT = nc.dram_tensor("xT", [n_steps * 4, BL], DT, kind="ExternalInput")
    w12t = nc.dram_tensor("w12t", [97, 4 * HS], DT, kind="ExternalInput")
    w1x = nc.dram_tensor("w1x", [4, 4 * HS], DT, kind="ExternalInput")
    wf1 = nc.dram_tensor("wf1", [97, 16], DT, kind="ExternalInput")
    wf2 = nc.dram_tensor("wf2", [16, 1], DT, kind="ExternalInput")
    bf2 = nc.dram_tensor("bf2", [1, 1], F32, kind="ExternalInput")
    out = nc.dram_tensor("out", [1, BL], F32, kind="ExternalOutput")

    with tile.TileContext(nc) as tc, ExitStack() as ctx:
        const = ctx.enter_context(tc.tile_pool(name="const", bufs=1))
        xpool = ctx.enter_context(tc.tile_pool(name="xp", bufs=8))
        gates = ctx.enter_context(tc.tile_pool(name="gates", bufs=3))

        W12 = const.tile([97, 4 * HS], DT, tag="W12")
        nc.sync.dma_start(W12[:], w12t.ap())
        W1X = const.tile([4, 4 * HS], DT, tag="W1X")
        nc.sync.dma_start(W1X[:], w1x.ap())
        WF1 = const.tile([97, 16], DT, tag="WF1")
        nc.sync.dma_start(WF1[:], wf1.ap())
        WF2 = const.tile([16, 1], DT, tag="WF2")
        nc.sync.dma_start(WF2[:], wf2.ap())
        BF2 = const.tile([1, 1], F32, tag="BF2")
        nc.sync.dma_start(BF2[:], bf2.ap())

        S = const.tile([97, BL], DT, tag="S")
        C = const.tile([HS, BL], CDT, tag="C")
        nc.vector.memset(S[:], 0.0)
        nc.vector.memset(S[96:97, :], 1.0)
        nc.vector.memset(C[:], 0.0)

        # Per-gate PSUM tiles (per-bank dependency tracking, so each
        # activation op starts as soon as its own gate's matmuls finish):
        # Pf [96,512] (f), Pio [96,1024] (i|o), Pg [96,512] (g).
        # Layer-1 (cols 0:64 of each gate's weight block) and layer-2
        # (cols 64:96) are fused into ONE M=96 matmul per gate — they
        # share the rhs S[0:97].  The x-projection is a K=4 matmul per
        # gate (M=96, layer-2 columns zero) emitted one step AHEAD
        # (start=True), so it fills the TensorE pipe during the previous
        # step's ACT/DVE chain; the recurrent matmul accumulates on top.
        #
        # Software-pipelined over layers: at iteration k the layer-1
        # partition computes LSTM-1 step k while the layer-2 partition
        # computes LSTM-2 step k-1 (both read h1_{k-1} from S).
        # Iteration 0 produces garbage layer-2 state (cleared after);
        # iteration n_steps produces garbage layer-1 state (the head
        # weights are zero over the h1 rows).
        GSEL = {"i": 0, "f": 1, "g": 2, "o": 3}

        def alloc_P():
            Pf = psum.tile([HS, BL], F32, tag="Pf")
            Pi = psum.tile([HS, BL], F32, tag="Pi")
            Pg = psum.tile([HS, BL], F32, tag="Pg")
            Po = psum.tile([HS, BL], F32, tag="Po")
            # (gate, dest-ap) in emission order: f, i, g, o —
            # f first (feeds f*c as early as possible), o last (only
            # needed at the very end for h = o * tanh(c)).
            return [
                ("f", Pf[:, :]),
                ("i", Pi[:, :]),
                ("g", Pg[:, :]),
                ("o", Po[:, :]),
            ], Pf, Pi, Pg, Po

        def emit_x_mms(banks, step):
            XTT = xpool.tile([4, BL], DT, tag="xt")
            nc.sync.dma_start(XTT[:], xT.ap()[4 * step : 4 * step + 4, :])
            for gate, dest in banks:
                gsel = GSEL[gate]
                nc.tensor.matmul(
                    dest,
                    W1X[:, gsel * HS : (gsel + 1) * HS],
                    XTT[:],
                    start=True,
                    stop=False,
                )

        with tc.tile_pool(name="psum", bufs=2, space="PSUM") as psum:
            banks, Pf, Pi, Pg, Po = alloc_P()
            emit_x_mms(banks, 0)
            for k in range(n_steps + 1):
                has_x = k < n_steps  # P already holds the x contribution
                for gate, dest in banks:
                    gsel = GSEL[gate]
                    nc.tensor.matmul(
                        dest,
                        W12[:, gsel * HS : (gsel + 1) * HS],
                        S[0:97, :],
                        start=not has_x,
                        stop=True,
                    )

                if k + 1 <= n_steps:
                    nbanks, nPf, nPi, nPg, nPo = alloc_P()
                    if k + 1 < n_steps:
                        emit_x_mms(nbanks, k + 1)

                SIGF = gates.tile([HS, BL], DT, tag="SIGF")
                SIGI = gates.tile([HS, BL], DT, tag="SIGI")
                G = gates.tile([HS, BL], DT, tag="G")
                SIGO = gates.tile([HS, BL], DT, tag="SIGO")
                nc.scalar.activation(SIGF[:], Pf[:, :], AF.Sigmoid)
                nc.scalar.activation(SIGI[:], Pi[:, :], AF.Sigmoid)
                nc.scalar.activation(G[:], Pg[:, :], AF.Tanh)
                nc.scalar.activation(SIGO[:], Po[:, :], AF.Sigmoid)

                U = gates.tile([HS, BL], DT, tag="U")
                V = gates.tile([HS, BL], CDT, tag="V")
                nc.vector.tensor_mul(V[:], SIGF[:], C[:])               # f*c
                nc.vector.tensor_mul(U[:], SIGI[:], G[:])               # i*g
                nc.vector.tensor_add(C[:], U[:], V[:])                  # c'
                TC = gates.tile([HS, BL], DT, tag="TC")
                nc.scalar.activation(TC[:], C[:], AF.Tanh)
                nc.vector.tensor_mul(S[0:HS, :], SIGO[:], TC[:])        # h
                if k == 0:
                    # wipe the garbage layer-2 state from the pipeline warmup
                    nc.vector.memset(S[H1:HS, :], 0.0)
                    nc.vector.memset(C[H1:HS, :], 0.0)
                if k + 1 <= n_steps:
                    banks, Pf, Pi, Pg, Po = nbanks, nPf, nPi, nPg, nPo

        # MLP head on h2 at the last timestep (rows 64:96 of S).
        with tc.tile_pool(name="psum_head", bufs=1, space="PSUM") as psh:
            PF = psh.tile([16, BL], F32, tag="PF")
            nc.tensor.matmul(PF[:], WF1[:, :], S[0:97, :], start=True, stop=True)
            Z = gates.tile([16, BL], DT, tag="Z")
            nc.scalar.activation(Z[:], PF[:], AF.Relu)
            PO = psh.tile([1, BL], F32, tag="PO")
            nc.tensor.matmul(PO[:], WF2[:, :], Z[:], start=True, stop=True)
            Y = gates.tile([1, BL], F32, tag="Y")
            nc.scalar.activation(Y[:], PO[:], AF.Identity, bias=BF2[:, 0:1])
            nc.sync.dma_start(out.ap(), Y[:])

    nc.compile()
    return nc


def _pack_weights(inputs, np_dt):
    w_ih1, w_hh1 = inputs["w_ih1"], inputs["w_hh1"]
    w_ih2, w_hh2 = inputs["w_ih2"], inputs["w_hh2"]
    b1 = (inputs["b_ih1"] + inputs["b_hh1"]).astype(np.float32)
    b2 = (inputs["b_ih2"] + inputs["b_hh2"]).astype(np.float32)
    # Layer-1 gate weights as [97, 256]: rows = [w_hh1^T(64); zeros(32);
    # bias1(1)] matching rhs S[0:97] = [h1; h2(ignored); ones].
    z32 = np.zeros((4 * H1, 32), np.float32)
    w1t = np.concatenate([w_hh1, z32, b1[:, None]], axis=1).T
    # Layer-2 gate weights as [97, 128]: rows = [w_ih2^T(64); w_hh2^T(32);
    # bias2(1)].
    w2t = np.concatenate([w_ih2, w_hh2, b2[:, None]], axis=1).T
    # Fused per-gate blocks [97, 96]: layer-1 output units in cols 0:64,
    # layer-2 in cols 64:96 (one M=96 matmul per gate).
    w12t = np.concatenate(
        [np.concatenate([w1t[:, g * H1 : (g + 1) * H1],
                         w2t[:, g * H2 : (g + 1) * H2]], axis=1)
         for g in range(4)], axis=1)
    # Input projection [4, 384]: per gate [w_ih1^T (64) | zeros (32)].
    zx = np.zeros((4, H2), np.float32)
    w1x = np.concatenate(
        [np.concatenate([w_ih1.T[:, g * H1 : (g + 1) * H1], zx], axis=1)
         for g in range(4)], axis=1)
    return {
        "w12t": np.ascontiguousarray(w12t).astype(np_dt),
        "w1x": np.ascontiguousarray(w1x).astype(np_dt),
        "wf1": np.ascontiguousarray(np.concatenate(
            [np.zeros((64, 16), np.float32), inputs["w_fc1"].T,
             inputs["b_fc1"][None, :]], axis=0)).astype(np_dt),
        "wf2": np.ascontiguousarray(inputs["w_fc2"].T).astype(np_dt),
        "bf2": np.ascontiguousarray(inputs["b_fc2"][:, None]).astype(np.float32),
    }


_built = {}


def _get_nc(n_steps):
    if n_steps not in _built:
        _built[n_steps] = _build(n_steps)
    return _built[n_steps]


def _run(inputs, n_steps=T, **run_kwargs):
    np_dt = mybir.dt.np(DT)
    x = np.asarray(inputs["x"], np.float32)
    nb = x.shape[0]
    ncores = NCORES
    bl = nb // ncores
    assert bl == BL and x.shape[1] >= n_steps
    shared = _pack_weights({k: np.asarray(v, np.float32) for k, v in inputs.items()
                            if k != "x"} | {}, np_dt)
    in_maps = []
    for c in range(ncores):
        xs = x[c * bl : (c + 1) * bl, :n_steps, :]  # [BL, T, 4]
        xT = np.ascontiguousarray(xs.transpose(1, 2, 0).reshape(n_steps * 4, bl))
        in_maps.append(dict(shared, xT=xT.astype(np_dt)))
    nc = _get_nc(n_steps)
    res = bass_utils.run_bass_kernel_spmd(
        nc, in_maps, core_ids=list(range(ncores)), **run_kwargs
    )
    y = np.concatenate(
        [np.asarray(r["out"], np.float32).reshape(bl, 1) for r in res.results], axis=0
    )
    return y, res


def kernel(**inputs) -> np.ndarray:
    y, _ = _run(inputs)
    return y



# revision 5
# speedup vs baseline: 1.3226x; 1.3226x over previous
"""Trainium2 Bass kernel for a 2-layer LSTM (64, 32) + MLP head.

Model (PyTorch semantics, eval mode):
    h1 = LSTM(4 -> 64)(x)            x: [B=4096, T=512, 4]
    h2 = LSTM(64 -> 32)(h1)
    y  = (relu(h2[:, -1] @ w_fc1.T + b_fc1)) @ w_fc2.T + b_fc2   # [B, 1]

Sharding: data-parallel over batch across 8 NeuronCores (512 rows each),
weights replicated.

Per-core schedule (v2):
  * State kept transposed and stacked: S [101, 256] per stream =
    [h1 (64); h2 (32); x_t (4); ones (1)].  Layer-1 and layer-2 are
    software-pipelined by one step and share the same rhs (layer-1's
    weight rows over h2 are zero, layer-2's over x are zero), so each
    gate needs ONE matmul (M=96 stacked units, K=101) -- 4 MMs/step,
    with the x projection folded into the rhs (no separate x matmuls).
  * The batch 512 is split into TWO independent streams of 256 that
    run phase-shifted; this hides the per-step serial chain
    (MM -> sigmoid -> cell ops -> tanh -> h) behind the other stream.
  * ALL four gates use sigmoid: tanh(z) = 2*sigmoid(2z) - 1, with the
    g-gate weight block pre-scaled by 2.  One ACTIVATE per stream-step
    covers all gates ([96, 1024] over 2 PSUM banks); a cheap DVE
    tensor_scalar (2*s-1) recovers g.  (ACT op cost ~= (N+350)/1.2 ns,
    so merging 4x256 gates into one op saves ~1.9us/step vs 5 ops.)
  * x_t arrives by DMA into rows 96:100 of the next S tile (3-deep
    rotation per stream) one step ahead.
"""

import numpy as np
from contextlib import ExitStack

import concourse.bass as bass
import concourse.tile as tile
from concourse import bacc, mybir
from concourse import bass_utils

AF = mybir.ActivationFunctionType
ALU = mybir.AluOpType

B, T, D_IN, H1, H2 = 4096, 512, 4, 64, 32
NCORES = 8
BL = B // NCORES        # 512 batch rows per core
NSTREAM = 2
SL = BL // NSTREAM      # 256 batch rows per stream

F32 = mybir.dt.float32
DT = mybir.dt.bfloat16

HS = H1 + H2            # 96 stacked units
KS = HS + D_IN + 1      # 101 rhs rows: h1|h2|x|ones
XROW = HS               # 96: first x row
ONEROW = HS + D_IN      # 100: ones row
R = 3                   # S-tile rotation depth per stream

# gate order in the fused weight/psum layout
GATES = ("i", "f", "g", "o")


def _build(n_steps: int = T):
    nc = bacc.Bacc("TRN2", target_bir_lowering=False, debug=False)

    xT = nc.dram_tensor("xT", [n_steps * 4, BL], DT, kind="ExternalInput")
    w12 = nc.dram_tensor("w12", [KS, 4 * HS], DT, kind="ExternalInput")
    wf1 = nc.dram_tensor("wf1", [KS, 16], DT, kind="ExternalInput")
    wf2 = nc.dram_tensor("wf2", [16, 1], DT, kind="ExternalInput")
    bf2 = nc.dram_tensor("bf2", [1, 1], F32, kind="ExternalInput")
    out = nc.dram_tensor("out", [1, BL], F32, kind="ExternalOutput")

    with tile.TileContext(nc) as tc, ExitStack() as ctx:
        const = ctx.enter_context(tc.tile_pool(name="const", bufs=1))
        gtp = ctx.enter_context(tc.tile_pool(name="gt", bufs=3))
        scr = ctx.enter_context(tc.tile_pool(name="scr", bufs=6))

        W12 = const.tile([KS, 4 * HS], DT, tag="W12")
        nc.sync.dma_start(W12[:], w12.ap())
        WF1 = const.tile([KS, 16], DT, tag="WF1")
        nc.sync.dma_start(WF1[:], wf1.ap())
        WF2 = const.tile([16, 1], DT, tag="WF2")
        nc.sync.dma_start(WF2[:], wf2.ap())
        BF2 = const.tile([1, 1], F32, tag="BF2")
        nc.sync.dma_start(BF2[:], bf2.ap())

        # Per-stream persistent state
        S = [[const.tile([KS, SL], DT, name=f"S{s}_{r}", tag=f"S{s}_{r}")
              for r in range(R)] for s in range(NSTREAM)]
        C = [const.tile([HS, SL], DT, name=f"C{s}", tag=f"C{s}")
             for s in range(NSTREAM)]
        for s in range(NSTREAM):
            for r in range(R):
                nc.vector.memset(S[s][r][:], 0.0)
                # base partition must be 32-aligned: set rows 96:101 to one;
                # the x DMA overwrites rows 96:100 before every use.
                nc.vector.memset(S[s][r][XROW:KS, :], 1.0)
            nc.vector.memset(C[s][:], 0.0)

        def dma_x(s, k):
            # x_k for stream s into rows 96:100 of S[s][k%R]
            if k < n_steps:
                nc.sync.dma_start(
                    S[s][k % R][XROW : XROW + D_IN, :],
                    xT.ap()[4 * k : 4 * k + 4, s * SL : (s + 1) * SL],
                )

        with tc.tile_pool(name="psum0", bufs=2, space="PSUM") as psum0, \
             tc.tile_pool(name="psum1", bufs=2, space="PSUM") as psum1:
            psums = [psum0, psum1]
            for s in range(NSTREAM):
                dma_x(s, 0)
                dma_x(s, 1)

            for k in range(n_steps + 1):
                for s in range(NSTREAM):
                    Scur = S[s][k % R]
                    Snxt = S[s][(k + 1) % R]
                    P = psums[s].tile([HS, 4 * SL], F32, tag=f"P{s}")
                    for g in range(4):
                        nc.tensor.matmul(
                            P[:, g * SL : (g + 1) * SL],
                            W12[:, g * HS : (g + 1) * HS],
                            Scur[:],
                            start=True,
                            stop=True,
                        )
                    # one sigmoid over all 4 gates (2 PSUM banks)
                    GT = gtp.tile([HS, 4 * SL], DT, tag=f"GT{s}")
                    nc.scalar.activation(GT[:], P[:], AF.Sigmoid)
                    SGI = GT[:, 0:SL]
                    SGF = GT[:, SL : 2 * SL]
                    SGG = GT[:, 2 * SL : 3 * SL]
                    SGO = GT[:, 3 * SL : 4 * SL]

                    GX = scr.tile([HS, SL], DT, tag=f"GX{s}")
                    # g = 2*sigmoid(2 z) - 1  (weights pre-scaled by 2)
                    nc.vector.tensor_scalar(GX[:], SGG, 2.0, -1.0, ALU.mult,
                                            ALU.add)
                    M1 = scr.tile([HS, SL], DT, tag=f"M1{s}")
                    nc.vector.tensor_tensor(M1[:], SGF, C[s][:], ALU.mult)
                    M2 = scr.tile([HS, SL], DT, tag=f"M2{s}")
                    nc.vector.tensor_tensor(M2[:], SGI, GX[:], ALU.mult)
                    nc.vector.tensor_tensor(C[s][:], M1[:], M2[:], ALU.add)
                    TC = scr.tile([HS, SL], DT, tag=f"TC{s}")
                    nc.scalar.activation(TC[:], C[s][:], AF.Tanh)
                    nc.vector.tensor_tensor(Snxt[0:HS, :], SGO, TC[:],
                                            ALU.mult)
                    if k == 0:
                        # wipe garbage layer-2 state from pipeline warmup
                        nc.vector.memset(Snxt[H1:HS, :], 0.0)
                        nc.vector.memset(C[s][H1:HS, :], 0.0)
                    dma_x(s, k + 2)

        # MLP head on h2 of the final state tiles
        with tc.tile_pool(name="psh", bufs=1, space="PSUM") as psh:
            for s in range(NSTREAM):
                Sfin = S[s][(n_steps + 1) % R]
                PF = psh.tile([16, SL], F32, tag=f"PF{s}")
                nc.tensor.matmul(PF[:], WF1[:, :], Sfin[:], start=True,
                                 stop=True)
                Z = scr.tile([16, SL], DT, tag=f"Z{s}")
                nc.scalar.activation(Z[:], PF[:], AF.Relu)
                PO = psh.tile([1, SL], F32, tag=f"PO{s}")
                nc.tensor.matmul(PO[:], WF2[:, :], Z[:], start=True, stop=True)
                Y = scr.tile([1, SL], F32, tag=f"Y{s}")
                nc.scalar.activation(Y[:], PO[:], AF.Identity,
                                     bias=BF2[:, 0:1])
                nc.sync.dma_start(out.ap()[:, s * SL : (s + 1) * SL], Y[:])

    nc.compile()
    return nc


def _pack_weights(inputs, np_dt):
    w_ih1, w_hh1 = inputs["w_ih1"], inputs["w_hh1"]
    w_ih2, w_hh2 = inputs["w_ih2"], inputs["w_hh2"]
    b1 = (inputs["b_ih1"] + inputs["b_hh1"]).astype(np.float32)
    b2 = (inputs["b_ih2"] + inputs["b_hh2"]).astype(np.float32)

    w12 = np.zeros((KS, 4 * HS), np.float32)
    for g in range(4):
        scale = 2.0 if g == 2 else 1.0  # g-gate: tanh(z) = 2 sig(2z) - 1
        c0 = g * HS
        # layer-1 units: cols c0 : c0+64
        w12[0:H1, c0 : c0 + H1] = w_hh1[g * H1 : (g + 1) * H1, :].T * scale
        w12[XROW : XROW + D_IN, c0 : c0 + H1] = (
            w_ih1[g * H1 : (g + 1) * H1, :].T * scale
        )
        w12[ONEROW, c0 : c0 + H1] = b1[g * H1 : (g + 1) * H1] * scale
        # layer-2 units: cols c0+64 : c0+96
        w12[0:H1, c0 + H1 : c0 + HS] = w_ih2[g * H2 : (g + 1) * H2, :].T * scale
        w12[H1:HS, c0 + H1 : c0 + HS] = w_hh2[g * H2 : (g + 1) * H2, :].T * scale
        w12[ONEROW, c0 + H1 : c0 + HS] = b2[g * H2 : (g + 1) * H2] * scale

    wf1 = np.zeros((KS, 16), np.float32)
    wf1[H1:HS, :] = inputs["w_fc1"].T
    wf1[ONEROW, :] = inputs["b_fc1"]

    return {
        "w12": np.ascontiguousarray(w12).astype(np_dt),
        "wf1": np.ascontiguousarray(wf1).astype(np_dt),
        "wf2": np.ascontiguousarray(inputs["w_fc2"].T).astype(np_dt),
        "bf2": np.ascontiguousarray(inputs["b_fc2"][:, None]).astype(np.float32),
    }


_built = {}


def _get_nc(n_steps):
    if n_steps not in _built:
        _built[n_steps] = _build(n_steps)
    return _built[n_steps]


def _run(inputs, n_steps=T, **run_kwargs):
    np_dt = mybir.dt.np(DT)
    x = np.asarray(inputs["x"], np.float32)
    nb = x.shape[0]
    ncores = NCORES
    bl = nb // ncores
    assert bl == BL and x.shape[1] >= n_steps
    shared = _pack_weights(
        {k: np.asarray(v, np.float32) for k, v in inputs.items() if k != "x"},
        np_dt,
    )
    in_maps = []
    for c in range(ncores):
        xs = x[c * bl : (c + 1) * bl, :n_steps, :]  # [BL, T, 4]
        xTc = np.ascontiguousarray(
            xs.transpose(1, 2, 0).reshape(n_steps * 4, bl)
        )
        in_maps.append(dict(shared, xT=xTc.astype(np_dt)))
    nc = _get_nc(n_steps)
    res = bass_utils.run_bass_kernel_spmd(
        nc, in_maps, core_ids=list(range(ncores)), **run_kwargs
    )
    y = np.concatenate(
        [np.asarray(r["out"], np.float32).reshape(bl, 1) for r in res.results],
        axis=0,
    )
    return y, res


def kernel(**inputs) -> np.ndarray:
    y, _ = _run(inputs)
    return y
